# revision 28
# baseline (speedup 1.0000x reference)
"""Spiking-NN classifier (LIF over T=25 steps) on 8 Trainium2 NeuronCores.

Strategy: pure data parallel over the batch (4096 -> 512 rows/core), tiny
weights replicated. Per core the kernel:
  1. computes the time-invariant layer currents with a chain of PE matmuls
     (everything kept in "transposed" [hidden, batch] orientation so no
     on-device transposes are needed),
  2. runs the LIF recurrence for 25 steps with work split across engines
     (DVE: decay + reset-subtract, GpSimd: spike compare, ACT: surrogate
     sigmoid),
  3. streams spk/soft outputs to HBM each step. Spikes are exactly 0/1 so
     they travel as uint8 (4x fewer bytes); the host casts back to f32.

Per-core DRAM output layouts are chosen for >=512B-contiguous DMA
descriptor runs; the host gather transposes them back to the reference
layout.
"""

import numpy as np

B = 4096
NCORES = 8
BC = B // NCORES          # 512 batch rows per core
L = 8                     # hidden LIF layers
T = 25
NIN = 784
KT = 7                    # k-tiles of 128 for the padded input dim
NINP = KT * 128           # 896, zero-padded input features
NH = 64
NO = 10
BETA = 0.95
THR = 1.0
NBB = BC // 128           # 4 batch blocks of 128 for the output layer
FREE = (L // 2) * BC      # 2048 free elems/partition of hidden state


def _build_nc(spk_u8=True, nchunk=4, out_engine="dve"):
    import concourse.bacc as bacc
    import concourse.bass as bass
    import concourse.mybir as mybir
    import concourse.tile as tile

    f32 = mybir.dt.float32
    u8 = mybir.dt.uint8
    spk_dt = u8 if spk_u8 else f32
    Alu = mybir.AluOpType
    Act = mybir.ActivationFunctionType
    assert nchunk == L // 2, "chunks are mapped 1:1 to layer-pair cur tiles"
    ch = FREE // nchunk

    nc = bacc.Bacc("TRN2", target_bir_lowering=False)

    xT = nc.dram_tensor("xT", [NINP, BC], f32, kind="ExternalInput")
    w0t = nc.dram_tensor("w0t", [NINP, NH], f32, kind="ExternalInput")
    # Small weights arrive pre-duplicated onto both partition halves (PE
    # quadrant tiling needs lhsT and rhs at the same base partition) so each
    # is a single contiguous DMA — HWDGE descriptor slots are the scarce
    # resource during the ramp.
    wht = nc.dram_tensor("wht", [128, L - 1, NH], f32, kind="ExternalInput")
    wot = nc.dram_tensor("wot", [128, NO], f32, kind="ExternalInput")
    bct = nc.dram_tensor("bct", [128, L], f32, kind="ExternalInput")
    bo = nc.dram_tensor("bo", [128, NO], f32, kind="ExternalInput")

    # Hidden outputs in transposed per-core layout [t, l, h, b]; partition
    # p = (l%2)*64 + h fuses to a linear stride so each DMA covers 4 layers.
    spkh = nc.dram_tensor("spkh", [T, L, NH, BC], spk_dt, kind="ExternalOutput")
    softh = nc.dram_tensor("softh", [T, L, NH, BC], f32, kind="ExternalOutput")
    # Output-layer results as [t, p, bb, h]; local batch row = bb*128 + p.
    spko = nc.dram_tensor("spko", [T, 128, NBB, NO], f32, kind="ExternalOutput")
    softo = nc.dram_tensor("softo", [T, 128, NBB, NO], f32, kind="ExternalOutput")

    with (
        tile.TileContext(nc) as tc,
        tc.tile_pool(name="const", bufs=1) as constp,
        tc.tile_pool(name="state", bufs=1) as statep,
        tc.tile_pool(name="outs", bufs=4) as outp,
        tc.tile_pool(name="psum", bufs=2, space=bass.MemorySpace.PSUM) as psump,
    ):
        nthr = constp.tile([128, 1], f32)
        nc.vector.memset(nthr[:], -THR)
        # Dummy sigmoid so the ACT LUT table loads overlap the input DMAs
        # instead of stalling the first real sigmoid.
        warm = constp.tile([128, 1], f32)
        nc.scalar.activation(warm[:], nthr[:], Act.Sigmoid, bias=nthr[:], scale=1.0)

        # Load order tuned for the serial ramp: w0t + first x chunk first
        # (they gate layer 0), the other small constants in the shadow of
        # the x transfer, the x tail last.
        xt_d = xT.rearrange("(kt p) b -> p kt b", p=128)
        XSPLIT = 2
        # Two x tiles (not one) so the k<XSPLIT matmuls don't wait on the
        # second DMA through coarse tile-granularity dependencies.
        xt_a = constp.tile([128, XSPLIT, BC], f32)
        xt_b = constp.tile([128, KT - XSPLIT, BC], f32)
        w0t_sb = constp.tile([128, KT, NH], f32)
        nc.sync.dma_start(w0t_sb[:], w0t.rearrange("(kt p) m -> p kt m", p=128))
        nc.scalar.dma_start(xt_a[:], xt_d[:, 0:XSPLIT, :])
        bct_sb = constp.tile([128, L], f32)
        nc.sync.dma_start(bct_sb[:], bct[:, :])
        bo_sb = constp.tile([128, NO], f32)
        nc.sync.dma_start(bo_sb[:], bo[:, :])
        wht_sb = constp.tile([128, L - 1, NH], f32)
        nc.sync.dma_start(wht_sb[:], wht[:, :, :])
        wot_sb = constp.tile([128, NO], f32)
        nc.sync.dma_start(wot_sb[:], wot[:, :])
        nc.scalar.dma_start(xt_b[:], xt_d[:, XSPLIT:KT, :])

        # Layer currents, transposed: layer l lives at partitions
        # [64*(l%2), 64*(l%2)+64) of tile l//2. One tile per layer pair so
        # early LIF steps can start while the chain is still running.
        cur = [
            statep.tile([128, BC], f32, name=f"cur{i}", tag=f"cur{i}")
            for i in range(L // 2)
        ]

        # Chain matmuls split into batch-halves so PE can run layer l+1's
        # first half while DVE copies layer l's second half out of PSUM.
        HB = BC // 2
        for l in range(L):
            e, lp = l % 2, l // 2
            for hb in range(2):
                bs = slice(hb * HB, (hb + 1) * HB)
                ps = psump.tile([128, HB], f32, tag="ps", bufs=4)
                od = ps[e * 64 : (e + 1) * 64, :]
                if l == 0:
                    for kt in range(KT):
                        xt_t = (
                            xt_a[:, kt, bs]
                            if kt < XSPLIT
                            else xt_b[:, kt - XSPLIT, bs]
                        )
                        nc.tensor.matmul(
                            od,
                            w0t_sb[:, kt, :],
                            xt_t,
                            start=(kt == 0),
                            stop=(kt == KT - 1),
                        )
                else:
                    ep = (l - 1) % 2
                    rhs = cur[(l - 1) // 2][ep * 64 : (ep + 1) * 64, bs]
                    lhsT = wht_sb[ep * 64 : (ep + 1) * 64, l - 1, :]
                    nc.tensor.matmul(od, lhsT, rhs, start=True, stop=True)
                nc.vector.tensor_scalar(
                    cur[lp][e * 64 : (e + 1) * 64, bs],
                    od,
                    bct_sb[e * 64 : (e + 1) * 64, l : l + 1],
                    None,
                    Alu.add,
                )

        # Output layer currents, batch-on-partition: curo[p, bb, :] is the
        # current for local batch row bb*128 + p.
        curo = statep.tile([128, NBB, NO], f32)
        pso = psump.tile([128, NBB, NO], f32, tag="pso")
        for bb in range(NBB):
            lhsT = cur[L // 2 - 1][64:128, bb * 128 : (bb + 1) * 128]
            nc.tensor.matmul(
                pso[:, bb, :], lhsT, wot_sb[NH:128, :], start=True, stop=True
            )
            nc.vector.tensor_add(curo[:, bb, :], pso[:, bb, :], bo_sb[:])

        curo_f = curo[:].rearrange("p bb h -> p (bb h)")

        ma = statep.tile([128, FREE], f32)      # membrane entering step t
        mb = statep.tile([128, FREE], f32)      # post-decay membrane
        nc.vector.memset(ma[:], 0.0)
        moa = statep.tile([128, NBB * NO], f32)
        mob = statep.tile([128, NBB * NO], f32)
        spko_acc = statep.tile([128, T, NBB * NO], f32)
        softo_acc = statep.tile([128, T, NBB * NO], f32)

        oeng = nc.gpsimd if out_engine == "pool" else nc.vector

        def hid_dst(dram, t, lp0, nlp):
            # [p=(e,h), lp, b] view of dram[t, 2*lp0 : 2*(lp0+nlp), :, :]
            off = t * L * NH * BC + 2 * lp0 * NH * BC
            return bass.AP(dram, off, [[BC, 128], [2 * NH * BC, nlp], [1, BC]])

        for t in range(T):
            spk = outp.tile([128, FREE], spk_dt, tag="spk")
            soft = outp.tile([128, FREE], f32, tag="soft")
            # Post-decay membrane views, one per lp chunk. At t=0 the
            # membrane entering is zero, so it's just `cur` and the decay op
            # is skipped (this also lets LIF t=0 overlap the matmul chain).
            # Engine split per chunk: chunks 0,1 -> compare on Pool, sub on
            # DVE; chunks 2,3 -> compare on DVE (2x single-src mode), sub on
            # Pool. DVE emission interleaves the c2/c3 compares between the
            # decays so Pool's subs never stall on them.
            mbv = []
            for c in range(nchunk):
                s = slice(c * ch, (c + 1) * ch)
                if t == 0:
                    mbv.append(cur[c][:, :])
                else:
                    nc.vector.scalar_tensor_tensor(
                        mb[:, s], ma[:, s], BETA, cur[c][:, :], Alu.mult, Alu.add
                    )
                    mbv.append(mb[:, s])
                if c >= 2 and t > 0:
                    sc = slice(c * ch, (c + 1) * ch)
                    nc.vector.tensor_scalar(
                        spk[:, sc], mbv[c], THR, None, Alu.is_gt
                    )
            for c in range(nchunk):
                s = slice(c * ch, (c + 1) * ch)
                if c < 2:
                    nc.gpsimd.tensor_scalar(spk[:, s], mbv[c], THR, None, Alu.is_gt)
                elif t == 0:
                    # At t=0 cur[2]/cur[3] are the last chain outputs; emit
                    # these after the c0/c1 subs so DVE isn't order-blocked.
                    nc.vector.tensor_scalar(spk[:, s], mbv[c], THR, None, Alu.is_gt)
                nc.scalar.activation(
                    soft[:, s], mbv[c], Act.Sigmoid, bias=nthr[:], scale=1.0
                )
                if t < T - 1:
                    (nc.vector if c < 2 else nc.gpsimd).tensor_sub(
                        ma[:, s], mbv[c], spk[:, s]
                    )
            nc.sync.dma_start(
                hid_dst(spkh, t, 0, 4),
                spk[:].rearrange("p (lp b) -> p lp b", lp=4),
            )
            for h in range(4):
                nc.sync.dma_start(
                    hid_dst(softh, t, h, 1),
                    soft[:, h * FREE // 4 : (h + 1) * FREE // 4].rearrange(
                        "p (lp b) -> p lp b", lp=1
                    ),
                )

            # tiny output layer
            if t == 0:
                mobv = curo_f
            else:
                mobv = mob[:]
                oeng.scalar_tensor_tensor(
                    mob[:], moa[:], BETA, curo_f, Alu.mult, Alu.add
                )
            nc.gpsimd.tensor_scalar(spko_acc[:, t, :], mobv, THR, None, Alu.is_gt)
            nc.scalar.activation(
                softo_acc[:, t, :], mobv, Act.Sigmoid, bias=nthr[:], scale=1.0
            )
            if t < T - 1:
                nc.gpsimd.tensor_sub(moa[:], mobv, spko_acc[:, t, :])

            # Flush the first half of the accumulators mid-kernel so the
            # final drain isn't waiting on two fresh DMAs.
            if t == T // 2:
                h = T // 2 + 1
                oap = [[NBB * NO, 128], [128 * NBB * NO, h], [1, NBB * NO]]
                nc.sync.dma_start(bass.AP(spko, 0, oap), spko_acc[:, 0:h, :])
                nc.sync.dma_start(bass.AP(softo, 0, oap), softo_acc[:, 0:h, :])

        h = T // 2 + 1
        off = h * 128 * NBB * NO
        oap = [[NBB * NO, 128], [128 * NBB * NO, T - h], [1, NBB * NO]]
        nc.sync.dma_start(bass.AP(spko, off, oap), spko_acc[:, h:T, :])
        nc.sync.dma_start(bass.AP(softo, off, oap), softo_acc[:, h:T, :])

    nc.finalize()
    return nc


_NC_CACHE = []


def _get_nc():
    if not _NC_CACHE:
        _NC_CACHE.append(_build_nc())
    return _NC_CACHE[0]


def _prep_inputs(inputs):
    x = np.ascontiguousarray(np.asarray(inputs["x"], np.float32))
    W0 = np.asarray(inputs["W0"], np.float32)
    b0 = np.asarray(inputs["b0"], np.float32)
    Wh = np.asarray(inputs["Wh"], np.float32)
    bh = np.asarray(inputs["bh"], np.float32)
    Wo = np.asarray(inputs["Wo"], np.float32)
    bo = np.asarray(inputs["bo"], np.float32)

    xTp = np.zeros((NINP, B), np.float32)
    xTp[:NIN] = x.T
    w0tp = np.zeros((NINP, NH), np.float32)
    w0tp[:NIN] = W0.T
    # [k, i, m] with the 64 k-rows duplicated onto both partition halves
    whtk = Wh.transpose(2, 0, 1)  # [64, 7, 64]
    wht2 = np.ascontiguousarray(np.concatenate([whtk, whtk], axis=0))
    wot2 = np.ascontiguousarray(np.concatenate([Wo.T, Wo.T], axis=0))  # [128, 10]
    bct = np.concatenate([b0[:, None], bh.T], axis=1)  # [64, 8]
    bct2 = np.ascontiguousarray(np.concatenate([bct, bct], axis=0))  # [128, 8]
    bo2 = np.ascontiguousarray(np.broadcast_to(bo[None, :], (128, NO)))

    shared = {"w0t": w0tp, "wht": wht2, "wot": wot2, "bct": bct2, "bo": bo2}
    return [
        {"xT": np.ascontiguousarray(xTp[:, c * BC : (c + 1) * BC]), **shared}
        for c in range(NCORES)
    ]


def _gather(results):
    spk_h = np.empty((T, L, B, NH), np.float32)
    soft_h = np.empty((T, L, B, NH), np.float32)
    spk_o = np.empty((T, B, NO), np.float32)
    soft_o = np.empty((T, B, NO), np.float32)
    for c, r in enumerate(results):
        cs = slice(c * BC, (c + 1) * BC)
        spk_h[:, :, cs, :] = r["spkh"].astype(np.float32).transpose(0, 1, 3, 2)
        soft_h[:, :, cs, :] = r["softh"].transpose(0, 1, 3, 2)
        spk_o[:, cs, :] = (
            r["spko"].transpose(0, 2, 1, 3).reshape(T, BC, NO)
        )
        soft_o[:, cs, :] = (
            r["softo"].transpose(0, 2, 1, 3).reshape(T, BC, NO)
        )
    return spk_h, soft_h, spk_o, soft_o


def run(inputs, **spmd_kwargs):
    from concourse.bass_utils import run_bass_kernel_spmd

    nc = _get_nc()
    in_maps = _prep_inputs(inputs)
    res = run_bass_kernel_spmd(
        nc, in_maps, core_ids=list(range(NCORES)), **spmd_kwargs
    )
    return _gather(res.results), res


def kernel(**inputs):
    outputs, _ = run(inputs)
    return outputs



# revision 29
# speedup vs baseline: 1.1953x; 1.1953x over previous
"""Spiking-NN classifier (LIF over T=25 steps) on 8 Trainium2 NeuronCores.

Strategy: pure data parallel over the batch (4096 -> 512 rows/core), tiny
weights replicated. Per core the kernel:
  1. computes the time-invariant layer currents with a chain of PE matmuls
     (everything kept in "transposed" [hidden, batch] orientation so no
     on-device transposes are needed),
  2. runs the LIF recurrence for 25 steps with work split across engines
     (DVE: decay + reset-subtract, GpSimd: spike compare, ACT: surrogate
     sigmoid),
  3. streams spk/soft outputs to HBM each step. Spikes are exactly 0/1 so
     they travel as uint8 (4x fewer bytes); the host casts back to f32.

Per-core DRAM output layouts are chosen for >=512B-contiguous DMA
descriptor runs; the host gather transposes them back to the reference
layout.
"""

import numpy as np

B = 4096
NCORES = 8
BC = B // NCORES          # 512 batch rows per core
L = 8                     # hidden LIF layers
T = 25
NIN = 784
KT = 7                    # k-tiles of 128 for the padded input dim
NINP = KT * 128           # 896, zero-padded input features
NH = 64
NO = 10
BETA = 0.95
THR = 1.0
NBB = BC // 128           # 4 batch blocks of 128 for the output layer
FREE = (L // 2) * BC      # 2048 free elems/partition of hidden state


def _register_lif_op():
    """Fused LIF update as a custom DVE op (documented dve_ops.OPS API):
    out = (in0 - (in0 > 1)) * beta + in1 — one uop, line-rate, bit-exact
    match for the reference's subtract -> scale -> add sequence."""
    import concourse.dve_ops as dve_ops
    from concourse.dve_spec import Spec, Src0, Src1, C0, One, lower
    from concourse.dve_uop import DveOpSpec

    name = "LIF_STEP_ANT"
    if name in dve_ops._SUB_OPCODE_FOR_NAME:
        return next(op for op in dve_ops.OPS if op.name == name)
    spec = Spec(body=(Src0 - (Src0 > One)) * C0 + Src1)
    row = dve_ops._CUSTOM_DVE_ROW_BASE + len(dve_ops.OPS)
    assert row < 0x20
    shas = {}
    for ver in ("v3", "v4"):
        uops = lower(spec, ver=ver)
        assert len(uops) == 1, f"LIF spec no longer fits one uop on {ver}"
        shas[ver] = DveOpSpec(name=name, opcode=row, uops=uops, rd1_en=True).sha(ver)
    op = dve_ops.DveOp(name, spec, subdim=False, uops_sha=shas)
    dve_ops.OPS.append(op)
    dve_ops._SUB_OPCODE_FOR_NAME[name] = row
    return op


def _build_nc(spk_u8=True, nchunk=4, out_engine="dve"):
    import concourse.bacc as bacc
    import concourse.bass as bass
    import concourse.mybir as mybir
    import concourse.tile as tile

    f32 = mybir.dt.float32
    u8 = mybir.dt.uint8
    spk_dt = u8 if spk_u8 else f32
    Alu = mybir.AluOpType
    Act = mybir.ActivationFunctionType
    assert nchunk == L // 2, "chunks are mapped 1:1 to layer-pair cur tiles"
    ch = FREE // nchunk

    LIF = _register_lif_op()

    nc = bacc.Bacc("TRN2", target_bir_lowering=False)

    xT = nc.dram_tensor("xT", [NINP, BC], f32, kind="ExternalInput")
    w0t = nc.dram_tensor("w0t", [NINP, NH], f32, kind="ExternalInput")
    # Small weights arrive pre-duplicated onto both partition halves (PE
    # quadrant tiling needs lhsT and rhs at the same base partition) so each
    # is a single contiguous DMA — HWDGE descriptor slots are the scarce
    # resource during the ramp.
    wht = nc.dram_tensor("wht", [128, L - 1, NH], f32, kind="ExternalInput")
    wot = nc.dram_tensor("wot", [128, NO], f32, kind="ExternalInput")
    bct = nc.dram_tensor("bct", [128, L], f32, kind="ExternalInput")
    bo = nc.dram_tensor("bo", [128, NO], f32, kind="ExternalInput")

    # Hidden outputs in transposed per-core layout [t, l, h, b]; partition
    # p = (l%2)*64 + h fuses to a linear stride so each DMA covers 4 layers.
    spkh = nc.dram_tensor("spkh", [T, L, NH, BC], spk_dt, kind="ExternalOutput")
    bf16 = mybir.dt.bfloat16
    softh = nc.dram_tensor("softh", [T, L, NH, BC], bf16, kind="ExternalOutput")
    # Output-layer results as [t, p, bb, h]; local batch row = bb*128 + p.
    spko = nc.dram_tensor("spko", [T, 128, NBB, NO], f32, kind="ExternalOutput")
    softo = nc.dram_tensor("softo", [T, 128, NBB, NO], f32, kind="ExternalOutput")

    with (
        tile.TileContext(nc) as tc,
        tc.tile_pool(name="const", bufs=1) as constp,
        tc.tile_pool(name="state", bufs=1) as statep,
        tc.tile_pool(name="outs", bufs=4) as outp,
        tc.tile_pool(name="psum", bufs=2, space=bass.MemorySpace.PSUM) as psump,
    ):
        nthr = constp.tile([128, 1], f32)
        nc.vector.memset(nthr[:], -THR)
        # Dummy sigmoid so the ACT LUT table loads overlap the input DMAs
        # instead of stalling the first real sigmoid.
        warm = constp.tile([128, 1], f32)
        nc.scalar.activation(warm[:], nthr[:], Act.Sigmoid, bias=nthr[:], scale=1.0)

        # Load order tuned for the serial ramp: w0t + first x chunk first
        # (they gate layer 0), the other small constants in the shadow of
        # the x transfer, the x tail last.
        xt_d = xT.rearrange("(kt p) b -> p kt b", p=128)
        XSPLIT = 2
        # Two x tiles (not one) so the k<XSPLIT matmuls don't wait on the
        # second DMA through coarse tile-granularity dependencies.
        xt_a = constp.tile([128, XSPLIT, BC], f32)
        xt_b = constp.tile([128, KT - XSPLIT, BC], f32)
        w0t_sb = constp.tile([128, KT, NH], f32)
        nc.sync.dma_start(w0t_sb[:], w0t.rearrange("(kt p) m -> p kt m", p=128))
        nc.scalar.dma_start(xt_a[:], xt_d[:, 0:XSPLIT, :])
        bct_sb = constp.tile([128, L], f32)
        nc.sync.dma_start(bct_sb[:], bct[:, :])
        bo_sb = constp.tile([128, NO], f32)
        nc.sync.dma_start(bo_sb[:], bo[:, :])
        wht_sb = constp.tile([128, L - 1, NH], f32)
        nc.sync.dma_start(wht_sb[:], wht[:, :, :])
        wot_sb = constp.tile([128, NO], f32)
        nc.sync.dma_start(wot_sb[:], wot[:, :])
        nc.scalar.dma_start(xt_b[:], xt_d[:, XSPLIT:KT, :])

        # Layer currents, transposed: layer l lives at partitions
        # [64*(l%2), 64*(l%2)+64) of tile l//2. One tile per layer pair so
        # early LIF steps can start while the chain is still running.
        cur = [
            statep.tile([128, BC], f32, name=f"cur{i}", tag=f"cur{i}")
            for i in range(L // 2)
        ]

        # Chain matmuls split into batch-halves so PE can run layer l+1's
        # first half while DVE copies layer l's second half out of PSUM.
        HB = BC // 2
        for l in range(L):
            e, lp = l % 2, l // 2
            for hb in range(2):
                bs = slice(hb * HB, (hb + 1) * HB)
                ps = psump.tile([128, HB], f32, tag="ps", bufs=4)
                od = ps[e * 64 : (e + 1) * 64, :]
                if l == 0:
                    for kt in range(KT):
                        xt_t = (
                            xt_a[:, kt, bs]
                            if kt < XSPLIT
                            else xt_b[:, kt - XSPLIT, bs]
                        )
                        nc.tensor.matmul(
                            od,
                            w0t_sb[:, kt, :],
                            xt_t,
                            start=(kt == 0),
                            stop=(kt == KT - 1),
                        )
                else:
                    ep = (l - 1) % 2
                    rhs = cur[(l - 1) // 2][ep * 64 : (ep + 1) * 64, bs]
                    lhsT = wht_sb[ep * 64 : (ep + 1) * 64, l - 1, :]
                    nc.tensor.matmul(od, lhsT, rhs, start=True, stop=True)
                nc.vector.tensor_scalar(
                    cur[lp][e * 64 : (e + 1) * 64, bs],
                    od,
                    bct_sb[e * 64 : (e + 1) * 64, l : l + 1],
                    None,
                    Alu.add,
                )

        # Output layer currents, batch-on-partition: curo[p, bb, :] is the
        # current for local batch row bb*128 + p.
        curo = statep.tile([128, NBB, NO], f32)
        pso = psump.tile([128, NBB, NO], f32, tag="pso")
        for bb in range(NBB):
            lhsT = cur[L // 2 - 1][64:128, bb * 128 : (bb + 1) * 128]
            nc.tensor.matmul(
                pso[:, bb, :], lhsT, wot_sb[NH:128, :], start=True, stop=True
            )
            nc.vector.tensor_add(curo[:, bb, :], pso[:, bb, :], bo_sb[:])

        curo_f = curo[:].rearrange("p bb h -> p (bb h)")

        # Ping-pong pre-reset membrane buffers; the fused op re-derives the
        # reset from the previous step's pre-reset state, so no separate
        # post-reset tensor is needed.
        mbuf = [
            statep.tile([128, FREE], f32, name=f"mbuf{i}", tag=f"mbuf{i}")
            for i in range(2)
        ]
        mobuf = [
            statep.tile([128, NBB * NO], f32, name=f"mobuf{i}", tag=f"mobuf{i}")
            for i in range(2)
        ]
        spko_acc = statep.tile([128, T, NBB * NO], f32)
        softo_acc = statep.tile([128, T, NBB * NO], f32)

        def hid_dst(dram, t, lp0, nlp):
            # [p=(e,h), lp, b] view of dram[t, 2*lp0 : 2*(lp0+nlp), :, :]
            off = t * L * NH * BC + 2 * lp0 * NH * BC
            return bass.AP(dram, off, [[BC, 128], [2 * NH * BC, nlp], [1, BC]])

        for t in range(T):
            spk = outp.tile([128, FREE], spk_dt, tag="spk")
            soft = outp.tile([128, FREE], bf16, tag="soft")
            # Post-decay membrane views, one per lp chunk. At t=0 the
            # membrane entering is zero, so it's just `cur` and the decay op
            # is skipped (this also lets LIF t=0 overlap the matmul chain).
            # Membrane update: one fused custom-DVE op per chunk computes
            # (m - (m > 1)) * beta + cur — the previous step's
            # reset-by-subtraction folded into this step's decay, reading the
            # previous pre-reset membrane (cur itself at t=1). Compares
            # split 3 on Pool / 1 on DVE; sigmoids in 1024-wide ACT ops.
            if t > 0:
                dst = mbuf[(t - 1) % 2]
                for c in range(nchunk):
                    s = slice(c * ch, (c + 1) * ch)
                    nc.vector._custom_dve(
                        LIF, out=dst[:, s], in0=prev[c], in1=cur[c][:, :], s0=BETA
                    )
                mbv = [dst[:, c * ch : (c + 1) * ch] for c in range(nchunk)]
            else:
                mbv = [cur[c][:, :] for c in range(nchunk)]
            for c in range(3):
                s = slice(c * ch, (c + 1) * ch)
                nc.gpsimd.tensor_scalar(spk[:, s], mbv[c], THR, None, Alu.is_gt)
            nc.vector.tensor_scalar(
                spk[:, 3 * ch : 4 * ch], mbv[3], THR, None, Alu.is_gt
            )
            if t > 0:
                for hh in range(2):
                    s = slice(hh * 2 * ch, (hh + 1) * 2 * ch)
                    nc.scalar.activation(
                        soft[:, s], mbuf[(t - 1) % 2][:, s],
                        Act.Sigmoid, bias=nthr[:], scale=1.0,
                    )
            else:
                for c in range(nchunk):
                    s = slice(c * ch, (c + 1) * ch)
                    nc.scalar.activation(
                        soft[:, s], mbv[c], Act.Sigmoid, bias=nthr[:], scale=1.0
                    )
            prev = mbv
            nc.sync.dma_start(
                hid_dst(spkh, t, 0, 4),
                spk[:].rearrange("p (lp b) -> p lp b", lp=4),
            )
            for h in range(4):
                nc.sync.dma_start(
                    hid_dst(softh, t, h, 1),
                    soft[:, h * FREE // 4 : (h + 1) * FREE // 4].rearrange(
                        "p (lp b) -> p lp b", lp=1
                    ),
                )

            # tiny output layer, same fused update
            if t == 0:
                mobv = curo_f
            else:
                od = mobuf[(t - 1) % 2][:]
                nc.vector._custom_dve(LIF, out=od, in0=mobv, in1=curo_f, s0=BETA)
                mobv = od
            nc.gpsimd.tensor_scalar(spko_acc[:, t, :], mobv, THR, None, Alu.is_gt)
            nc.scalar.activation(
                softo_acc[:, t, :], mobv, Act.Sigmoid, bias=nthr[:], scale=1.0
            )

            # Flush the first half of the accumulators mid-kernel so the
            # final drain isn't waiting on two fresh DMAs.
            if t == T // 2:
                h = T // 2 + 1
                oap = [[NBB * NO, 128], [128 * NBB * NO, h], [1, NBB * NO]]
                nc.sync.dma_start(bass.AP(spko, 0, oap), spko_acc[:, 0:h, :])
                nc.sync.dma_start(bass.AP(softo, 0, oap), softo_acc[:, 0:h, :])

        h = T // 2 + 1
        off = h * 128 * NBB * NO
        oap = [[NBB * NO, 128], [128 * NBB * NO, T - h], [1, NBB * NO]]
        nc.sync.dma_start(bass.AP(spko, off, oap), spko_acc[:, h:T, :])
        nc.sync.dma_start(bass.AP(softo, off, oap), softo_acc[:, h:T, :])

    nc.finalize()
    return nc


_NC_CACHE = []


def _get_nc():
    if not _NC_CACHE:
        _NC_CACHE.append(_build_nc())
    return _NC_CACHE[0]


def _prep_inputs(inputs):
    x = np.ascontiguousarray(np.asarray(inputs["x"], np.float32))
    W0 = np.asarray(inputs["W0"], np.float32)
    b0 = np.asarray(inputs["b0"], np.float32)
    Wh = np.asarray(inputs["Wh"], np.float32)
    bh = np.asarray(inputs["bh"], np.float32)
    Wo = np.asarray(inputs["Wo"], np.float32)
    bo = np.asarray(inputs["bo"], np.float32)

    xTp = np.zeros((NINP, B), np.float32)
    xTp[:NIN] = x.T
    w0tp = np.zeros((NINP, NH), np.float32)
    w0tp[:NIN] = W0.T
    # [k, i, m] with the 64 k-rows duplicated onto both partition halves
    whtk = Wh.transpose(2, 0, 1)  # [64, 7, 64]
    wht2 = np.ascontiguousarray(np.concatenate([whtk, whtk], axis=0))
    wot2 = np.ascontiguousarray(np.concatenate([Wo.T, Wo.T], axis=0))  # [128, 10]
    bct = np.concatenate([b0[:, None], bh.T], axis=1)  # [64, 8]
    bct2 = np.ascontiguousarray(np.concatenate([bct, bct], axis=0))  # [128, 8]
    bo2 = np.ascontiguousarray(np.broadcast_to(bo[None, :], (128, NO)))

    shared = {"w0t": w0tp, "wht": wht2, "wot": wot2, "bct": bct2, "bo": bo2}
    return [
        {"xT": np.ascontiguousarray(xTp[:, c * BC : (c + 1) * BC]), **shared}
        for c in range(NCORES)
    ]


def _gather(results):
    spk_h = np.empty((T, L, B, NH), np.float32)
    soft_h = np.empty((T, L, B, NH), np.float32)
    spk_o = np.empty((T, B, NO), np.float32)
    soft_o = np.empty((T, B, NO), np.float32)
    for c, r in enumerate(results):
        cs = slice(c * BC, (c + 1) * BC)
        spk_h[:, :, cs, :] = r["spkh"].astype(np.float32).transpose(0, 1, 3, 2)
        soft_h[:, :, cs, :] = r["softh"].astype(np.float32).transpose(0, 1, 3, 2)
        spk_o[:, cs, :] = (
            r["spko"].transpose(0, 2, 1, 3).reshape(T, BC, NO)
        )
        soft_o[:, cs, :] = (
            r["softo"].transpose(0, 2, 1, 3).reshape(T, BC, NO)
        )
    return spk_h, soft_h, spk_o, soft_o


def run(inputs, **spmd_kwargs):
    from concourse.bass_utils import run_bass_kernel_spmd

    nc = _get_nc()
    in_maps = _prep_inputs(inputs)
    res = run_bass_kernel_spmd(
        nc, in_maps, core_ids=list(range(NCORES)), **spmd_kwargs
    )
    return _gather(res.results), res


def kernel(**inputs):
    outputs, _ = run(inputs)
    return outputs



# revision 30
# speedup vs baseline: 1.3050x; 1.0918x over previous
"""Spiking-NN classifier (LIF over T=25 steps) on 8 Trainium2 NeuronCores.

Strategy: pure data parallel over the batch (4096 -> 512 rows/core), tiny
weights replicated. Per core the kernel:
  1. computes the time-invariant layer currents with a chain of PE matmuls
     (everything kept in "transposed" [hidden, batch] orientation so no
     on-device transposes are needed),
  2. runs the LIF recurrence for 25 steps with work split across engines
     (DVE: decay + reset-subtract, GpSimd: spike compare, ACT: surrogate
     sigmoid),
  3. streams spk/soft outputs to HBM each step. Spikes are exactly 0/1 so
     they travel as uint8 (4x fewer bytes); the host casts back to f32.

Per-core DRAM output layouts are chosen for >=512B-contiguous DMA
descriptor runs; the host gather transposes them back to the reference
layout.
"""

import numpy as np

B = 4096
NCORES = 8
BC = B // NCORES          # 512 batch rows per core
L = 8                     # hidden LIF layers
T = 25
NIN = 784
KT = 7                    # k-tiles of 128 for the padded input dim
NINP = KT * 128           # 896, zero-padded input features
NH = 64
NO = 10
BETA = 0.95
THR = 1.0
NBB = BC // 128           # 4 batch blocks of 128 for the output layer
FREE = (L // 2) * BC      # 2048 free elems/partition of hidden state


def _register_lif_op():
    """Fused LIF update as a custom DVE op (documented dve_ops.OPS API):
    out = (in0 - (in0 > 1)) * beta + in1 — one uop, line-rate, bit-exact
    match for the reference's subtract -> scale -> add sequence."""
    import concourse.dve_ops as dve_ops
    from concourse.dve_spec import Spec, Src0, Src1, C0, One, lower
    from concourse.dve_uop import DveOpSpec

    name = "LIF_STEP_ANT"
    if name in dve_ops._SUB_OPCODE_FOR_NAME:
        return next(op for op in dve_ops.OPS if op.name == name)
    spec = Spec(body=(Src0 - (Src0 > One)) * C0 + Src1)
    row = dve_ops._CUSTOM_DVE_ROW_BASE + len(dve_ops.OPS)
    assert row < 0x20
    shas = {}
    for ver in ("v3", "v4"):
        uops = lower(spec, ver=ver)
        assert len(uops) == 1, f"LIF spec no longer fits one uop on {ver}"
        shas[ver] = DveOpSpec(name=name, opcode=row, uops=uops, rd1_en=True).sha(ver)
    op = dve_ops.DveOp(name, spec, subdim=False, uops_sha=shas)
    dve_ops.OPS.append(op)
    dve_ops._SUB_OPCODE_FOR_NAME[name] = row
    return op


def _build_nc(spk_u8=True, nchunk=4, out_engine="dve"):
    import concourse.bacc as bacc
    import concourse.bass as bass
    import concourse.mybir as mybir
    import concourse.tile as tile

    f32 = mybir.dt.float32
    u8 = mybir.dt.uint8
    spk_dt = u8 if spk_u8 else f32
    Alu = mybir.AluOpType
    Act = mybir.ActivationFunctionType
    assert nchunk == L // 2, "chunks are mapped 1:1 to layer-pair cur tiles"
    ch = FREE // nchunk

    LIF = _register_lif_op()

    nc = bacc.Bacc("TRN2", target_bir_lowering=False)

    xT = nc.dram_tensor("xT", [NINP, BC], f32, kind="ExternalInput")
    w0t = nc.dram_tensor("w0t", [NINP, NH], f32, kind="ExternalInput")
    # Small weights arrive pre-duplicated onto both partition halves (PE
    # quadrant tiling needs lhsT and rhs at the same base partition) so each
    # is a single contiguous DMA — HWDGE descriptor slots are the scarce
    # resource during the ramp.
    wht = nc.dram_tensor("wht", [128, L - 1, NH], f32, kind="ExternalInput")
    wot = nc.dram_tensor("wot", [128, NO], f32, kind="ExternalInput")
    bct = nc.dram_tensor("bct", [128, L], f32, kind="ExternalInput")
    bo = nc.dram_tensor("bo", [128, NO], f32, kind="ExternalInput")

    # Hidden outputs in transposed per-core layout [t, l, h, b]; partition
    # p = (l%2)*64 + h fuses to a linear stride so each DMA covers 4 layers.
    spkh = nc.dram_tensor("spkh", [T, L, NH, BC], spk_dt, kind="ExternalOutput")
    bf16 = mybir.dt.bfloat16
    softh = nc.dram_tensor("softh", [T, L, NH, BC], bf16, kind="ExternalOutput")
    # Output-layer results as [t, p, bb, h]; local batch row = bb*128 + p.
    spko = nc.dram_tensor("spko", [T, 128, NBB, NO], f32, kind="ExternalOutput")
    softo = nc.dram_tensor("softo", [T, 128, NBB, NO], f32, kind="ExternalOutput")

    with (
        tile.TileContext(nc) as tc,
        tc.tile_pool(name="const", bufs=1) as constp,
        tc.tile_pool(name="state", bufs=1) as statep,
        tc.tile_pool(name="outs", bufs=4) as outp,
        tc.tile_pool(name="psum", bufs=2, space=bass.MemorySpace.PSUM) as psump,
    ):
        nthr = constp.tile([128, 1], f32)
        nc.vector.memset(nthr[:], -THR)
        # Dummy sigmoid so the ACT LUT table loads overlap the input DMAs
        # instead of stalling the first real sigmoid.
        warm = constp.tile([128, 1], f32)
        nc.scalar.activation(warm[:], nthr[:], Act.Sigmoid, bias=nthr[:], scale=1.0)

        # Load order tuned for the serial ramp: w0t + first x chunk first
        # (they gate layer 0), the other small constants in the shadow of
        # the x transfer, the x tail last.
        xt_d = xT.rearrange("(kt p) b -> p kt b", p=128)
        XSPLIT = 2
        # Two x tiles (not one) so the k<XSPLIT matmuls don't wait on the
        # second DMA through coarse tile-granularity dependencies.
        xt_a = constp.tile([128, XSPLIT, BC], f32)
        xt_b = constp.tile([128, KT - XSPLIT, BC], f32)
        w0t_sb = constp.tile([128, KT, NH], f32)
        nc.sync.dma_start(w0t_sb[:], w0t.rearrange("(kt p) m -> p kt m", p=128))
        nc.scalar.dma_start(xt_a[:], xt_d[:, 0:XSPLIT, :])
        bct_sb = constp.tile([128, L], f32)
        nc.sync.dma_start(bct_sb[:], bct[:, :])
        bo_sb = constp.tile([128, NO], f32)
        nc.sync.dma_start(bo_sb[:], bo[:, :])
        wht_sb = constp.tile([128, L - 1, NH], f32)
        nc.sync.dma_start(wht_sb[:], wht[:, :, :])
        wot_sb = constp.tile([128, NO], f32)
        nc.sync.dma_start(wot_sb[:], wot[:, :])
        nc.scalar.dma_start(xt_b[:], xt_d[:, XSPLIT:KT, :])

        # Layer currents, transposed: layer l lives at partitions
        # [64*(l%2), 64*(l%2)+64) of tile l//2. One tile per layer pair so
        # early LIF steps can start while the chain is still running.
        cur = [
            statep.tile([128, BC], f32, name=f"cur{i}", tag=f"cur{i}")
            for i in range(L // 2)
        ]

        # Chain matmuls split into batch-halves so PE can run layer l+1's
        # first half while DVE copies layer l's second half out of PSUM.
        HB = BC // 2
        for l in range(L):
            e, lp = l % 2, l // 2
            for hb in range(2):
                bs = slice(hb * HB, (hb + 1) * HB)
                ps = psump.tile([128, HB], f32, tag="ps", bufs=4)
                od = ps[e * 64 : (e + 1) * 64, :]
                if l == 0:
                    for kt in range(KT):
                        xt_t = (
                            xt_a[:, kt, bs]
                            if kt < XSPLIT
                            else xt_b[:, kt - XSPLIT, bs]
                        )
                        nc.tensor.matmul(
                            od,
                            w0t_sb[:, kt, :],
                            xt_t,
                            start=(kt == 0),
                            stop=(kt == KT - 1),
                        )
                else:
                    ep = (l - 1) % 2
                    rhs = cur[(l - 1) // 2][ep * 64 : (ep + 1) * 64, bs]
                    lhsT = wht_sb[ep * 64 : (ep + 1) * 64, l - 1, :]
                    nc.tensor.matmul(od, lhsT, rhs, start=True, stop=True)
                nc.vector.tensor_scalar(
                    cur[lp][e * 64 : (e + 1) * 64, bs],
                    od,
                    bct_sb[e * 64 : (e + 1) * 64, l : l + 1],
                    None,
                    Alu.add,
                )

        # Output layer currents, batch-on-partition: curo[p, bb, :] is the
        # current for local batch row bb*128 + p.
        curo = statep.tile([128, NBB, NO], f32)
        pso = psump.tile([128, NBB, NO], f32, tag="pso")
        for bb in range(NBB):
            lhsT = cur[L // 2 - 1][64:128, bb * 128 : (bb + 1) * 128]
            nc.tensor.matmul(
                pso[:, bb, :], lhsT, wot_sb[NH:128, :], start=True, stop=True
            )
            nc.vector.tensor_add(curo[:, bb, :], pso[:, bb, :], bo_sb[:])

        curo_f = curo[:].rearrange("p bb h -> p (bb h)")

        # Ping-pong pre-reset membrane buffers; the fused op re-derives the
        # reset from the previous step's pre-reset state, so no separate
        # post-reset tensor is needed.
        mbuf = [
            statep.tile([128, FREE], f32, name=f"mbuf{i}", tag=f"mbuf{i}")
            for i in range(2)
        ]
        mobuf = [
            statep.tile([128, NBB * NO], f32, name=f"mobuf{i}", tag=f"mobuf{i}")
            for i in range(2)
        ]
        spko_acc = statep.tile([128, T, NBB * NO], f32)
        softo_acc = statep.tile([128, T, NBB * NO], f32)

        def hid_dst(dram, t, lp0, nlp):
            # [p=(e,h), lp, b] view of dram[t, 2*lp0 : 2*(lp0+nlp), :, :]
            off = t * L * NH * BC + 2 * lp0 * NH * BC
            return bass.AP(dram, off, [[BC, 128], [2 * NH * BC, nlp], [1, BC]])

        for t in range(T):
            spk = outp.tile([128, FREE], spk_dt, tag="spk")
            soft = outp.tile([128, FREE], bf16, tag="soft")
            # Post-decay membrane views, one per lp chunk. At t=0 the
            # membrane entering is zero, so it's just `cur` and the decay op
            # is skipped (this also lets LIF t=0 overlap the matmul chain).
            # Membrane update: one fused custom-DVE op per chunk computes
            # (m - (m > 1)) * beta + cur — the previous step's
            # reset-by-subtraction folded into this step's decay, reading the
            # previous pre-reset membrane (cur itself at t=1). Compares
            # split 3 on Pool / 1 on DVE; sigmoids in 1024-wide ACT ops.
            if t > 0:
                dst = mbuf[(t - 1) % 2]
                for c in range(nchunk):
                    s = slice(c * ch, (c + 1) * ch)
                    nc.vector._custom_dve(
                        LIF, out=dst[:, s], in0=prev[c], in1=cur[c][:, :], s0=BETA
                    )
                mbv = [dst[:, c * ch : (c + 1) * ch] for c in range(nchunk)]
            else:
                mbv = [cur[c][:, :] for c in range(nchunk)]
            for c in range(3):
                s = slice(c * ch, (c + 1) * ch)
                nc.gpsimd.tensor_scalar(spk[:, s], mbv[c], THR, None, Alu.is_gt)
            nc.vector.tensor_scalar(
                spk[:, 3 * ch : 4 * ch], mbv[3], THR, None, Alu.is_gt
            )
            if t > 0:
                for hh in range(2):
                    s = slice(hh * 2 * ch, (hh + 1) * 2 * ch)
                    nc.scalar.activation(
                        soft[:, s], mbuf[(t - 1) % 2][:, s],
                        Act.Sigmoid, bias=nthr[:], scale=1.0,
                    )
            else:
                for c in range(nchunk):
                    s = slice(c * ch, (c + 1) * ch)
                    nc.scalar.activation(
                        soft[:, s], mbv[c], Act.Sigmoid, bias=nthr[:], scale=1.0
                    )
            prev = mbv
            # One store per tensor per step, on separate HWDGE rings —
            # descriptor generation (625ns serial per dma_start per ring) is
            # the store-path pacer, not DMA bandwidth.
            nc.sync.dma_start(
                hid_dst(spkh, t, 0, 4),
                spk[:].rearrange("p (lp b) -> p lp b", lp=4),
            )
            nc.scalar.dma_start(
                hid_dst(softh, t, 0, 4),
                soft[:].rearrange("p (lp b) -> p lp b", lp=4),
            )

            # tiny output layer, same fused update
            if t == 0:
                mobv = curo_f
            else:
                od = mobuf[(t - 1) % 2][:]
                nc.vector._custom_dve(LIF, out=od, in0=mobv, in1=curo_f, s0=BETA)
                mobv = od
            nc.gpsimd.tensor_scalar(spko_acc[:, t, :], mobv, THR, None, Alu.is_gt)
            nc.scalar.activation(
                softo_acc[:, t, :], mobv, Act.Sigmoid, bias=nthr[:], scale=1.0
            )

            # Flush the first half of the accumulators mid-kernel so the
            # final drain isn't waiting on two fresh DMAs.
            if t == T // 2:
                h = T // 2 + 1
                oap = [[NBB * NO, 128], [128 * NBB * NO, h], [1, NBB * NO]]
                nc.sync.dma_start(bass.AP(spko, 0, oap), spko_acc[:, 0:h, :])
                nc.sync.dma_start(bass.AP(softo, 0, oap), softo_acc[:, 0:h, :])

        h = T // 2 + 1
        off = h * 128 * NBB * NO
        oap = [[NBB * NO, 128], [128 * NBB * NO, T - h], [1, NBB * NO]]
        nc.sync.dma_start(bass.AP(spko, off, oap), spko_acc[:, h:T, :])
        nc.sync.dma_start(bass.AP(softo, off, oap), softo_acc[:, h:T, :])

    nc.finalize()
    return nc


_NC_CACHE = []


def _get_nc():
    if not _NC_CACHE:
        _NC_CACHE.append(_build_nc())
    return _NC_CACHE[0]


def _prep_inputs(inputs):
    x = np.ascontiguousarray(np.asarray(inputs["x"], np.float32))
    W0 = np.asarray(inputs["W0"], np.float32)
    b0 = np.asarray(inputs["b0"], np.float32)
    Wh = np.asarray(inputs["Wh"], np.float32)
    bh = np.asarray(inputs["bh"], np.float32)
    Wo = np.asarray(inputs["Wo"], np.float32)
    bo = np.asarray(inputs["bo"], np.float32)

    xTp = np.zeros((NINP, B), np.float32)
    xTp[:NIN] = x.T
    w0tp = np.zeros((NINP, NH), np.float32)
    w0tp[:NIN] = W0.T
    # [k, i, m] with the 64 k-rows duplicated onto both partition halves
    whtk = Wh.transpose(2, 0, 1)  # [64, 7, 64]
    wht2 = np.ascontiguousarray(np.concatenate([whtk, whtk], axis=0))
    wot2 = np.ascontiguousarray(np.concatenate([Wo.T, Wo.T], axis=0))  # [128, 10]
    bct = np.concatenate([b0[:, None], bh.T], axis=1)  # [64, 8]
    bct2 = np.ascontiguousarray(np.concatenate([bct, bct], axis=0))  # [128, 8]
    bo2 = np.ascontiguousarray(np.broadcast_to(bo[None, :], (128, NO)))

    shared = {"w0t": w0tp, "wht": wht2, "wot": wot2, "bct": bct2, "bo": bo2}
    return [
        {"xT": np.ascontiguousarray(xTp[:, c * BC : (c + 1) * BC]), **shared}
        for c in range(NCORES)
    ]


def _gather(results):
    spk_h = np.empty((T, L, B, NH), np.float32)
    soft_h = np.empty((T, L, B, NH), np.float32)
    spk_o = np.empty((T, B, NO), np.float32)
    soft_o = np.empty((T, B, NO), np.float32)
    for c, r in enumerate(results):
        cs = slice(c * BC, (c + 1) * BC)
        spk_h[:, :, cs, :] = r["spkh"].astype(np.float32).transpose(0, 1, 3, 2)
        soft_h[:, :, cs, :] = r["softh"].astype(np.float32).transpose(0, 1, 3, 2)
        spk_o[:, cs, :] = (
            r["spko"].transpose(0, 2, 1, 3).reshape(T, BC, NO)
        )
        soft_o[:, cs, :] = (
            r["softo"].transpose(0, 2, 1, 3).reshape(T, BC, NO)
        )
    return spk_h, soft_h, spk_o, soft_o


def run(inputs, **spmd_kwargs):
    from concourse.bass_utils import run_bass_kernel_spmd

    nc = _get_nc()
    in_maps = _prep_inputs(inputs)
    res = run_bass_kernel_spmd(
        nc, in_maps, core_ids=list(range(NCORES)), **spmd_kwargs
    )
    return _gather(res.results), res


def kernel(**inputs):
    outputs, _ = run(inputs)
    return outputs

